# revision 9
# baseline (speedup 1.0000x reference)
"""Single-head causal attention (B=8, T=2048, C=1024) on 8 trn2 NeuronCores.

Strategy: data-parallel over batch — one batch element per core, zero
communication. Per core:
  Q^T = Wq @ x^T   (scaled by C^-0.5),  K^T = Wk @ x^T,  V = x @ Wv^T
  S^T[k,q] = sum_h K^T[h,k] Q^T[h,q]    (k on partitions, q on free dim)
  P = exp(S^T) with causal mask via affine_select (k > q -> 0)
  denom[q] = P^T ones   (matmul against a ones vector, per q-tile)
  out[q,h] = (P^T)^T @ V / denom        (lhsT = P tiles, rhs = V natural)

All matmuls run as float32r (TF32-like; measured ~149ns for
[128,128]x[128,256], which beats one N=512 at ~330ns, so most matmuls are
issued 256-wide). PSUM->SBUF moves run on DVE; ACT only does exp.
Host-side prep: x and W are passed pre-transposed (weight prepacking) since
the fp32 DMA-transpose path doesn't exist on trn2; Q^T is spilled to a DRAM
scratch and streamed back per 512-wide q-chunk to fit SBUF.
"""

import numpy as np

import concourse.mybir as mybir
import concourse.tile as tile
from concourse import bacc
from concourse.bass_utils import run_bass_kernel_spmd

B, T, C = 8, 2048, 1024
QCH = 512          # q-chunk width (and t-chunk width in projection pass)
F32 = mybir.dt.float32
F32R = mybir.dt.float32r


def build_program(t_seq=T, phases=(1, 1, 1), reps=1):
    """Build the per-core Bass program. t_seq must be a multiple of QCH.

    reps > 1 wraps the whole body in a hardware loop (for timing runs)."""
    n_ch = t_seq // QCH          # q-chunks
    n_kt = t_seq // 128          # k-tiles
    scale = 1.0 / np.sqrt(C)

    nc = bacc.Bacc("TRN2", target_bir_lowering=False, debug=False)

    xt = nc.declare_dram_parameter("xt", [C, t_seq], F32, isOutput=False)
    wqt = nc.declare_dram_parameter("wqt", [C, C], F32, isOutput=False)
    wkt = nc.declare_dram_parameter("wkt", [C, C], F32, isOutput=False)
    wvt = nc.declare_dram_parameter("wvt", [C, C], F32, isOutput=False)
    out = nc.declare_dram_parameter("out", [t_seq, C], F32, isOutput=True)
    qt_dram = nc.dram_tensor("qt_spill", [C, t_seq], F32R)

    xt_r = xt[:, :].rearrange("(cc p) t -> p cc t", p=128)
    wqt_r = wqt[:, :].rearrange("(cc p) h -> p cc h", p=128)
    wkt_r = wkt[:, :].rearrange("(cc p) h -> p cc h", p=128)
    wvt_r = wvt[:, :].rearrange("(cc p) h -> p cc h", p=128)
    qt_dram_r = qt_dram[:, :].rearrange("(hc p) t -> p hc t", p=128)

    def body(tc):
        with tc.tile_pool(name="persist", bufs=1) as persist:
            kt_sb = persist.tile([128, 8, t_seq], F32R, tag="kt")
            ones_sb = persist.tile([128, 2], F32R, tag="ones")
            ones_f32 = persist.tile([128, 2], F32, tag="ones_f32")
            nc.vector.memset(ones_f32, 1.0)
            nc.vector.tensor_copy(out=ones_sb, in_=ones_f32)

            # ---- Pass 1: Q^T (spill to DRAM) and K^T (resident) ----
            if phases[0]:
                with tc.tile_pool(name="w1", bufs=1) as w1, \
                     tc.tile_pool(name="xin", bufs=2) as xin, \
                     tc.tile_pool(name="qstage", bufs=3) as qstage, \
                     tc.tile_pool(name="ps_a", bufs=4, space="PSUM") as ps_a:
                    wq_sb = w1.tile([128, 8, C], F32R, tag="wq")
                    wk_sb = w1.tile([128, 8, C], F32R, tag="wk")
                    xtcs = [xin.tile([128, 8, QCH], F32R, tag="xtc",
                                     name=f"xtc{i}") for i in range(n_ch)]
                    # first x chunk before the weight loads: the first
                    # matmuls need xtc0 + first half of wq only
                    nc.gpsimd.dma_start(
                        out=xtcs[0], in_=xt_r[:, :, 0:QCH])
                    nc.gpsimd.dma_start(out=wq_sb[:, :, 0:512],
                                        in_=wqt_r[:, :, 0:512])
                    nc.gpsimd.dma_start(out=wq_sb[:, :, 512:C],
                                        in_=wqt_r[:, :, 512:C])
                    nc.gpsimd.dma_start(out=wk_sb[:, :, 0:512],
                                        in_=wkt_r[:, :, 0:512])
                    nc.gpsimd.dma_start(out=wk_sb[:, :, 512:C],
                                        in_=wkt_r[:, :, 512:C])
                    for tc_i in range(n_ch):
                        xtc = xtcs[tc_i]
                        if tc_i > 0:
                            nc.gpsimd.dma_start(
                                out=xtc,
                                in_=xt_r[:, :, tc_i * QCH:(tc_i + 1) * QCH])
                        for ht in range(8):
                            hsl = slice(ht * 128, (ht + 1) * 128)
                            ps_q = ps_a.tile([128, QCH], F32, tag="psa")
                            ps_k = ps_a.tile([128, QCH], F32, tag="psa")
                            for cc in range(8):
                                nc.tensor.matmul(
                                    ps_q, wq_sb[:, cc, hsl], xtc[:, cc, :],
                                    start=(cc == 0), stop=(cc == 7))
                                nc.tensor.matmul(
                                    ps_k, wk_sb[:, cc, hsl], xtc[:, cc, :],
                                    start=(cc == 0), stop=(cc == 7))
                            qst = qstage.tile([128, QCH], F32R, tag="qst")
                            nc.scalar.activation(
                                qst, ps_q, mybir.ActivationFunctionType.Copy,
                                scale=float(scale))
                            nc.vector.tensor_copy(
                                out=kt_sb[:, ht,
                                          tc_i * QCH:(tc_i + 1) * QCH],
                                in_=ps_k)
                            nc.sync.dma_start(
                                out=qt_dram[hsl,
                                            tc_i * QCH:(tc_i + 1) * QCH],
                                in_=qst)

            # ---- Pass 2: V (resident, natural [t, h] layout) ----
            with tc.tile_pool(name="vpool", bufs=1) as vpool:
                v_sb = vpool.tile([128, n_kt, C], F32R, tag="v")
                if phases[1]:
                    with tc.tile_pool(name="w2", bufs=1) as w2, \
                         tc.tile_pool(name="xin2", bufs=2) as xin2, \
                         tc.tile_pool(name="ps_b", bufs=4, space="PSUM") \
                            as ps_b:
                        wv_sb = w2.tile([128, 8, C], F32R, tag="wv")
                        for ws in range(4):
                            nc.gpsimd.dma_start(
                                out=wv_sb[:, :, ws * 256:(ws + 1) * 256],
                                in_=wvt_r[:, :, ws * 256:(ws + 1) * 256])
                        for tg in range(n_kt // 2):   # two t-tiles per load
                            xtt = xin2.tile([128, 8, 256], F32R, tag="xtt")
                            nc.gpsimd.dma_start(
                                out=xtt,
                                in_=xt_r[:, :, tg * 256:(tg + 1) * 256])
                            for tj in range(2):
                                tt = tg * 2 + tj
                                for hc in range(2):
                                    hql = slice(hc * 512, (hc + 1) * 512)
                                    ps_v = ps_b.tile([128, 512], F32,
                                                     tag="psb")
                                    for cc in range(8):
                                        nc.tensor.matmul(
                                            ps_v,
                                            xtt[:, cc,
                                                tj * 128:(tj + 1) * 128],
                                            wv_sb[:, cc, hql],
                                            start=(cc == 0), stop=(cc == 7))
                                    nc.vector.tensor_copy(
                                        out=v_sb[:, tt, hql], in_=ps_v)

                # ---- Pass 3: per q-chunk S^T -> exp/mask -> PV + denom ----
                if phases[2]:
                    with tc.tile_pool(name="qt_in", bufs=1) as qt_in, \
                         tc.tile_pool(name="ptil", bufs=1) as ptil, \
                         tc.tile_pool(name="ostage", bufs=2) as ostage, \
                         tc.tile_pool(name="small", bufs=4) as small, \
                         tc.tile_pool(name="ps_s", bufs=2, space="PSUM") \
                            as ps_s, \
                         tc.tile_pool(name="ps_o", bufs=4, space="PSUM") \
                            as ps_o, \
                         tc.tile_pool(name="ps_d", bufs=2, space="PSUM") \
                            as ps_d:
                        for qc in range(n_ch):
                            q0 = qc * QCH
                            nk = (q0 + QCH) // 128  # k-tiles (causal)
                            qtc = qt_in.tile([128, 8, QCH], F32R, tag="qtc")
                            nc.sync.dma_start(
                                out=qtc, in_=qt_dram_r[:, :, q0:q0 + QCH])
                            p_sb = ptil.tile([128, n_kt, QCH], F32R, tag="p")
                            for tk in range(nk):
                                ps_st = ps_s.tile([128, QCH], F32, tag="pss")
                                for hc in range(8):
                                    nc.tensor.matmul(
                                        ps_st,
                                        kt_sb[:, hc,
                                              tk * 128:(tk + 1) * 128],
                                        qtc[:, hc, :],
                                        start=(hc == 0), stop=(hc == 7))
                                nc.scalar.activation(
                                    p_sb[:, tk, :], ps_st,
                                    mybir.ActivationFunctionType.Exp)
                                if 128 * tk + 127 > q0:
                                    nc.gpsimd.affine_select(
                                        out=p_sb[:, tk, :],
                                        in_=p_sb[:, tk, :],
                                        pattern=[[1, QCH]],
                                        base=q0 - 128 * tk,
                                        channel_multiplier=-1,
                                        compare_op=mybir.AluOpType.is_ge,
                                        fill=0.0)
                            for j in range(QCH // 128):
                                qtile = qc * (QCH // 128) + j
                                ps_o0 = ps_o.tile([128, 512], F32, tag="pso")
                                ps_o1 = ps_o.tile([128, 512], F32, tag="pso")
                                ps_den = ps_d.tile([128, 2], F32, tag="psd")
                                for tk in range(qtile + 1):
                                    p_t = p_sb[:, tk, j * 128:(j + 1) * 128]
                                    st = (tk == 0)
                                    sp = (tk == qtile)
                                    nc.tensor.matmul(ps_den, p_t, ones_sb,
                                                     start=st, stop=sp)
                                    nc.tensor.matmul(ps_o0, p_t,
                                                     v_sb[:, tk, 0:512],
                                                     start=st, stop=sp)
                                    nc.tensor.matmul(ps_o1, p_t,
                                                     v_sb[:, tk, 512:1024],
                                                     start=st, stop=sp)
                                recip = small.tile([128, 1], F32, tag="recip")
                                nc.vector.reciprocal(recip, ps_den[:, 0:1])
                                ost = ostage.tile([128, C], F32, tag="ost")
                                nc.vector.tensor_scalar_mul(
                                    ost[:, 0:512], ps_o0, recip)
                                nc.vector.tensor_scalar_mul(
                                    ost[:, 512:1024], ps_o1, recip)
                                nc.sync.dma_start(
                                    out=out[qtile * 128:(qtile + 1) * 128, :],
                                    in_=ost)

    with tile.TileContext(nc) as tc:
        if reps > 1:
            with tc.For_i(0, reps, 1):
                body(tc)
        else:
            body(tc)

    nc.compile()
    return nc


_nc_cache = {}


def _get_program(t_seq):
    if t_seq not in _nc_cache:
        _nc_cache[t_seq] = build_program(t_seq)
    return _nc_cache[t_seq]


def make_in_maps(x, Wk, Wq, Wv):
    wqt = np.ascontiguousarray(Wq.T)
    wkt = np.ascontiguousarray(Wk.T)
    wvt = np.ascontiguousarray(Wv.T)
    return [
        {"xt": np.ascontiguousarray(x[b].T), "wqt": wqt, "wkt": wkt,
         "wvt": wvt}
        for b in range(x.shape[0])
    ]


def kernel(x, Wk, Wq, Wv):
    x = np.asarray(x, dtype=np.float32)
    nc = _get_program(x.shape[1])
    in_maps = make_in_maps(x, np.asarray(Wk, dtype=np.float32),
                           np.asarray(Wq, dtype=np.float32),
                           np.asarray(Wv, dtype=np.float32))
    res = run_bass_kernel_spmd(nc, in_maps, core_ids=list(range(x.shape[0])))
    return np.stack([res.results[b]["out"] for b in range(x.shape[0])])
